# revision 28
# baseline (speedup 1.0000x reference)
"""Trainium2 Bass kernel for masked-softmax attention scoring.

Reference computation (B=128, T=512, K=1024, Q=1024):
    mids  = einsum("kq,bq->bk", W, query)
    s     = tanh(einsum("btk,bk->bt", key, mids) + bias)
    attn  = softmax-like: exp(s - max) * mask / sum(exp(s - max) * mask)

The max-subtraction cancels exactly in the ratio (tanh is bounded), so the
device computes  attn = exp(tanh(.)) * mask / sum_t(exp(tanh(.)) * mask).

Sharding: data-parallel over B across 8 NeuronCores (16 batches/core).
Per-core layout: partition p = (b, j) with b in [0,16), j in [0,8);
free column c in [0,64); timestep t = j*64 + c.

The mids matmul writes the (b, j)-replicated layout directly: the
stationary operand is query^T with each batch column replicated 8x via a
stride-0 broadcast AP, so out[p, k] = mids[b(p), k].  W^T is streamed as
float32r (full-rate fp32 on the PE).  Scores are 64 fused multiply-reduce
DVE ops (affine_mul_reduce) against key chunks streamed on both HWDGE
rings; a tiny qw-dependent copy holds the scalar ring back so the W^T
prologue gets full HBM bandwidth.  Softmax normalization does the
8-partition group sum with a block-diagonal 0/1 matmul.
"""

import sys

if "/opt/trn_rl_repo" not in sys.path:
    sys.path.insert(0, "/opt/trn_rl_repo")

from contextlib import ExitStack

import numpy as np

# ---- problem constants (hardcoded per spec) ----
B, T, K, Q = 128, 512, 1024, 1024
NCORES = 8
BS = B // NCORES          # 16 batches per core
P = 128                   # SBUF partitions
J = P // BS               # 8 t-blocks per batch on partitions
CF = T // J               # 64 timesteps per (partition, free col)
CC = 4                    # t-cols per key DMA super-chunk (2 MB each)
NCH = CF // CC            # 16 key DMAs per core
QC = Q // P               # 8 contraction chunks for the mids matmul
KEY_BUFS = 7              # key tile pool depth

_STATE: dict = {}


def _build_nc():
    import concourse.tile as tile
    from concourse import bacc, mybir

    f32 = mybir.dt.float32
    f32r = mybir.dt.float32r
    nc = bacc.Bacc()

    qt_e = nc.declare_dram_parameter("qt", [P, QC, BS], f32r, isOutput=False)
    wt_e = nc.declare_dram_parameter("wt", [P, QC, K], f32r, isOutput=False)
    grp_e = nc.declare_dram_parameter("grp", [P, P], f32, isOutput=False)
    key_e = nc.declare_dram_parameter("key", [BS, T, K], f32, isOutput=False)
    maskr_e = nc.declare_dram_parameter("maskr", [P, CF], f32, isOutput=False)
    bias_e = nc.declare_dram_parameter("biasb", [P, 1], f32, isOutput=False)
    out_e = nc.declare_dram_parameter("out", [P, CF], f32, isOutput=True)

    with tile.TileContext(nc) as tc, ExitStack() as ctx:
        const = ctx.enter_context(tc.tile_pool(name="const", bufs=1))
        kpool = ctx.enter_context(tc.tile_pool(name="key", bufs=KEY_BUFS))
        spool = ctx.enter_context(tc.tile_pool(name="scratch", bufs=2))
        psum = ctx.enter_context(tc.tile_pool(name="psum", bufs=1, space="PSUM"))

        # ---- prologue loads on the sync ring: qt, then W^T chunks ----
        qt_sb = const.tile([P, QC, BS], f32r)
        nc.sync.dma_start(out=qt_sb[:], in_=qt_e[:])
        wt_sb = const.tile([P, QC, K], f32r)
        for qc in range(QC):
            nc.sync.dma_start(out=wt_sb[:, qc, :], in_=wt_e[:, qc, :])
        grp_sb = const.tile([P, P], f32)
        nc.sync.dma_start(out=grp_sb[:], in_=grp_e[:])
        maskr_sb = const.tile([P, CF], f32)
        nc.sync.dma_start(out=maskr_sb[:], in_=maskr_e[:])
        bias_sb = const.tile([P, 1], f32)
        nc.sync.dma_start(out=bias_sb[:], in_=bias_e[:])

        # ---- mids in broadcast layout: [P, K], row p = mids[b(p), :] ----
        # Replicate each query column 8x on-chip (stride-0 DVE read) so the
        # stationary operand has the (b, j) partition order in one free dim.
        qtrep_sb = const.tile([P, QC, BS, J], f32r)
        nc.vector.tensor_copy(
            qtrep_sb[:], qt_sb[:].unsqueeze(-1).broadcast_to((P, QC, BS, J))
        )
        mids_ps = psum.tile([P, K], f32)
        for qc in range(QC):
            lhsT = qtrep_sb[:, qc, :, :]
            for h in range(2):
                nc.tensor.matmul(
                    mids_ps[:, h * 512 : (h + 1) * 512],
                    lhsT=lhsT,
                    rhs=wt_sb[:, qc, h * 512 : (h + 1) * 512],
                    start=(qc == 0),
                    stop=(qc == QC - 1),
                )
        mids_bc = const.tile([P, K], f32)
        nc.vector.tensor_copy(mids_bc[:], mids_ps[:])

        # ---- scores[p, c] = key[b, j*64+c, :] . mids[b, :] ----
        # HBM is the wall (~330 GB/s aggregate, ~165 GB/s per HWDGE ring),
        # so both FIFO rings must carry key bytes end-to-end.  The sync ring
        # holds the W^T prologue plus 7 key chunks (18.2 MB); the scalar
        # ring streams 9 key chunks (18 MB) from t=0.  The DVE consumes
        # chunks in merged arrival order (ring chunks arrive every ~12 us
        # per ring; sync's first key chunk lands only after W^T), and each
        # chunk sources the score columns matching its consumption slot.
        NB = 9                       # chunks on the scalar ring
        NA = NCH - NB                # chunks on the sync ring (after W^T)
        arrivals = sorted(
            [("B", i, 1.0 + 12.1 * i) for i in range(NB)]
            + [("A", j, 26.0 + 12.1 * j) for j in range(NA)],
            key=lambda x: x[2],
        )
        order_of = {
            (ring, idx): n for n, (ring, idx, _) in enumerate(arrivals)
        }
        scores_sb = const.tile([P, CF], f32)
        key_r = key_e[:].rearrange("b (j c) k -> (b j) c k", j=J)
        ring_eng = {"A": nc.sync, "B": nc.scalar}
        ring_of = {order_of[(r, i)]: r for (r, i, _) in arrivals}
        ktiles = {}
        for n in range(NCH):
            kt = kpool.tile([P, CC, K], f32, tag="ktile")
            ring_eng[ring_of[n]].dma_start(
                out=kt[:], in_=key_r[:, n * CC : (n + 1) * CC, :]
            )
            ktiles[n] = kt
        for n in range(NCH):
            kt = ktiles[n]
            for cc in range(CC):
                c = n * CC + cc
                prod = spool.tile([P, K], f32, tag="prod")
                nc.vector.affine_mul_reduce(
                    out=prod[:],
                    accum_out=scores_sb[:, c : c + 1],
                    in0=kt[:, cc, :],
                    in1=mids_bc[:],
                    scale=1.0,
                    bias=0.0,
                )

        # ---- epilogue: tanh, exp, mask, normalize ----
        tanh_sb = const.tile([P, CF], f32)
        nc.scalar.activation(
            out=tanh_sb[:],
            in_=scores_sb[:],
            func=mybir.ActivationFunctionType.Tanh,
            bias=bias_sb[:],
            scale=1.0,
        )
        exp_sb = const.tile([P, CF], f32)
        nc.scalar.activation(
            out=exp_sb[:], in_=tanh_sb[:], func=mybir.ActivationFunctionType.Exp
        )
        em_sb = const.tile([P, CF], f32)
        rowsum = const.tile([P, 1], f32)
        nc.vector.affine_mul_reduce(
            out=em_sb[:],
            accum_out=rowsum[:],
            in0=exp_sb[:],
            in1=maskr_sb[:],
            scale=1.0,
            bias=0.0,
        )
        den_ps = psum.tile([P, 1], f32)
        nc.tensor.matmul(
            den_ps[:], lhsT=grp_sb[:], rhs=rowsum[:], start=True, stop=True
        )
        rinv = const.tile([P, 1], f32)
        nc.vector.reciprocal(out=rinv[:], in_=den_ps[:])
        attn_sb = const.tile([P, CF], f32)
        nc.vector.tensor_scalar_mul(attn_sb[:], em_sb[:], rinv[:])
        nc.scalar.dma_start(out=out_e[:], in_=attn_sb[:])

    nc.compile()
    return nc


def _get_nc():
    if "nc" not in _STATE:
        _STATE["nc"] = _build_nc()
    return _STATE["nc"]


def _grp():
    if "GRP" not in _STATE:
        # GRP[p, m] = 1 iff p // J == m // J  (block-diagonal group-sum)
        pj = np.arange(P) // J
        _STATE["GRP"] = np.ascontiguousarray(
            (pj[:, None] == pj[None, :]).astype(np.float32)
        )
    return _STATE["GRP"]


def _make_in_maps(query, key, mask, W, bias):
    query = np.asarray(query, dtype=np.float32)
    key = np.asarray(key, dtype=np.float32)
    mask = np.asarray(mask, dtype=np.float32)
    W = np.asarray(W, dtype=np.float32)
    bias = np.asarray(bias, dtype=np.float32).reshape(-1)

    # wt[p, qc, k] = W.T[qc*128 + p, k]
    WT = np.ascontiguousarray(
        np.ascontiguousarray(W.T).reshape(QC, P, K).transpose(1, 0, 2)
    )
    GRP = _grp()
    biasb = np.ascontiguousarray(
        np.broadcast_to(bias[:1][None, :], (P, 1)).astype(np.float32)
    )

    in_maps = []
    for i in range(NCORES):
        sh = slice(i * BS, (i + 1) * BS)
        in_maps.append(
            {
                # pre-laid [P, QC, BS]: qt[p, qc, b] = query[sh].T[qc*128+p, b]
                "qt": np.ascontiguousarray(
                    query[sh].T.reshape(QC, P, BS).transpose(1, 0, 2)
                ),
                "wt": WT,
                "grp": GRP,
                "key": np.ascontiguousarray(key[sh]),
                "maskr": np.ascontiguousarray(mask[sh]).reshape(P, CF),
                "biasb": biasb,
            }
        )
    return in_maps


def _run(in_maps, **kwargs):
    from concourse.bass_utils import run_bass_kernel_spmd

    return run_bass_kernel_spmd(
        _get_nc(), in_maps, core_ids=list(range(NCORES)), **kwargs
    )


def _gather(results):
    return np.concatenate(
        [np.asarray(r["out"]).reshape(BS, T) for r in results], axis=0
    )


def kernel(query, key, mask, W, bias):
    in_maps = _make_in_maps(query, key, mask, W, bias)
    res = _run(in_maps)
    return _gather(res.results)
